# revision 6
# baseline (speedup 1.0000x reference)
"""CapsuleLayer dynamic-routing kernel v2 for Trainium2 (8 NeuronCores).

Data-parallel over batch B (64 -> 8 per core), zero communication.

Layout: U[p=(n16,b8), kc, (d,j)] bf16 in SBUF; psum cols are (d,j) so
drains are contiguous. s0 (uniform-c weighted sum) accumulates on the PE
during phase 1 via a dense-x stationary. Routing sweeps are block-
pipelined: agree (TT mul + d-tree adds) -> softmax (per-kc exp with
fused Z-accumulation on Act, recip, norm) -> weight-mul -> s-matmul on
PE. All tensor-tensor work stays on DVE (real GpSimd is far slower than
the cost model claims); Act handles drains, exp, square/ln/exp squash
from one pinned activation table.
"""

import numpy as np
import ml_dtypes

from concourse import bass
import concourse.mybir as mybir
import concourse.bacc as bacc
import concourse.tile as tile
from concourse.bass_utils import run_bass_kernel_spmd

BF16 = mybir.dt.bfloat16
F32 = mybir.dt.float32
AF = mybir.ActivationFunctionType
ALU = mybir.AluOpType
AX = mybir.AxisListType

B, N, I, J, D = 64, 2048, 8, 32, 16
NCORES = 8
BL = B // NCORES          # 8 local batches
KC = N // 16              # 128 contraction chunks of 16 n's
JD = J * D                # 512
GRP = 4                   # kc's per DMA batch
NG = KC // GRP            # 32
BLK = 8                   # kc's per routing block
NBLK = KC // BLK          # 16
ZBUFS = 2
EPS = 1e-7


POOL_BLOCKS = ()
DRAIN_PATTERN = "vsvs"
ONCHIP_XBD = False
WM_FP8 = False
EXP_ACCUM = True
POOL_Y = ()


def _build_nc(reps=1):
    nc = bacc.Bacc("TRN2", target_bir_lowering=False)
    if ONCHIP_XBD:
        xbd_d = None
        msk_d = nc.declare_dram_parameter("msk", [128, 128], BF16, False)
    else:
        xbd_d = nc.declare_dram_parameter("xbd", [NG, 128, GRP, 128], BF16, False)
        msk_d = None
    xs_d = nc.declare_dram_parameter("xs", [128, NG, GRP, BL], BF16, False)
    wm_d = nc.declare_dram_parameter(
        "wm", [NG, 128, GRP, JD], mybir.dt.float8e4 if WM_FP8 else BF16, False
    )
    ones_d = nc.declare_dram_parameter("onesbd", [128, BL], BF16, False)
    bc8_d = nc.declare_dram_parameter("bcast8", [BL, 128], BF16, False)
    vout_d = nc.declare_dram_parameter("vout", [BL, JD], F32, True)

    for _ in range(reps):
        _emit_body(nc, xbd_d, msk_d, xs_d, wm_d, ones_d, bc8_d, vout_d)
    nc.compile()
    return nc


def _stt_add(eng, out, a, b):
    eng.tensor_add(out, a, b)


def _emit_body(nc, xbd_d, msk_d, xs_d, wm_d, ones_d, bc8_d, vout_d):
    with tile.TileContext(nc) as tc:
        with (
            tc.tile_pool(name="big", bufs=1) as big,
            tc.tile_pool(name="wpool", bufs=3) as wpool,
            tc.tile_pool(name="xpool", bufs=3) as xpool,
            tc.tile_pool(name="ppool", bufs=4, space="PSUM") as ppool,
            tc.tile_pool(name="spool", bufs=1, space="PSUM") as spool,
            tc.tile_pool(name="vpool", bufs=1, space="PSUM") as vpsum,
            tc.tile_pool(name="zpool", bufs=ZBUFS) as zpool,
            tc.tile_pool(name="small", bufs=1) as small,
        ):
            # persistent SBUF
            U = big.tile([128, KC, D, J], BF16, tag="U")
            blg = big.tile([128, KC, J], BF16, tag="blg")
            onesbd = small.tile([128, BL], BF16, tag="ones")
            xs = small.tile([128, NG, GRP, BL], BF16, tag="xs")
            bc8 = small.tile([BL, 128], BF16, tag="bc8")
            vrep = small.tile([128, D, J], BF16, tag="vrep")
            epst = small.tile([128, 1], F32, tag="epst")
            nc.vector.memset(epst[:], EPS)
            # pin the act table to natural_log_exp_and_others (ln+exp+square
            # +copy) so no reloads happen mid-kernel
            actwarm = small.tile([128, 1], F32, tag="actwarm")
            nc.scalar.activation(actwarm[:], epst[:], AF.Ln)
            if ONCHIP_XBD:
                msk = small.tile([128, 128], BF16, tag="msk")
                nc.sync.dma_start(out=msk[:], in_=msk_d[:])
            nc.sync.dma_start(out=onesbd[:], in_=ones_d[:])
            nc.sync.dma_start(out=bc8[:], in_=bc8_d[:])
            nc.sync.dma_start(out=xs[:], in_=xs_d[:])

            # ---- Phase A: u_hat + s0 accumulation ----
            spt0 = spool.tile([BL, JD], F32, tag="s0")
            drain_eng = [{"v": nc.vector, "s": nc.scalar}[c] for c in DRAIN_PATTERN]
            for g in range(NG):
                wt = wpool.tile(
                    [128, GRP, JD], mybir.dt.float8e4 if WM_FP8 else BF16, tag="w"
                )
                xt = xpool.tile([128, GRP, 128], BF16, tag="x")
                nc.sync.dma_start(out=wt[:], in_=wm_d[g])
                if ONCHIP_XBD:
                    xsin = (
                        xs[:, g, :, :]
                        .unsqueeze(2)
                        .broadcast_to([128, GRP, 16, BL])
                    )
                    mskin = (
                        msk[:]
                        .rearrange("p (n b) -> p n b", n=16, b=BL)
                        .unsqueeze(1)
                        .broadcast_to([128, GRP, 16, BL])
                    )
                    xtv = xt[:].rearrange("p q (n b) -> p q n b", n=16, b=BL)
                    nc.vector.tensor_mul(xtv, xsin, mskin)
                else:
                    nc.sync.dma_start(out=xt[:], in_=xbd_d[g])
                for q in range(GRP):
                    kc = g * GRP + q
                    pt = ppool.tile([128, JD], F32, tag="p1")
                    nc.tensor.matmul(
                        pt[:], lhsT=xt[:, q, :], rhs=wt[:, q, :],
                        start=True, stop=True,
                    )
                    nc.tensor.matmul(
                        spt0[:], lhsT=xs[:, g, q, :], rhs=wt[:, q, :],
                        start=(kc == 0), stop=(kc == KC - 1),
                    )
                    eng = drain_eng[kc % 4]
                    dst = U[:, kc, :, :].rearrange("p d j -> p (d j)")
                    if eng is nc.scalar:
                        eng.copy(dst, pt[:])
                    else:
                        eng.tensor_copy(dst, pt[:])

            # ---- routing ----
            for it in range(3):
                # squash s -> v; v in psum row-broadcast to vrep
                spt = spt0 if it == 0 else spt_i
                cscale = (1.0 / J) if it == 0 else 1.0
                # sq[b, j, d] = (s * cscale)^2 straight from PSUM on Act
                sq = small.tile([BL, J, D], F32, tag="sq")
                src_jd = spt[:].rearrange("p (d j) -> p j d", d=D, j=J)
                nc.scalar.activation(sq[:], src_jd, AF.Square, scale=cscale)
                ssq = small.tile([BL, J], F32, tag="ssq")
                nc.vector.tensor_reduce(ssq[:], sq[:], axis=AX.X, op=ALU.add)
                # rden = (ssq + eps)^-0.5 via ln/exp (same act table as Exp)
                lnq = small.tile([BL, J], F32, tag="lnq")
                nc.scalar.activation(lnq[:], ssq[:], AF.Ln, bias=epst[0:BL, :])
                rden = small.tile([BL, J], F32, tag="rden")
                nc.scalar.activation(rden[:], lnq[:], AF.Exp, scale=-0.5)

                if it == 2:
                    # final output v[b, j, d] fp32 = s * cscale * rden
                    vf = small.tile([BL, J, D], F32, tag="vf")
                    den_in = rden[:].unsqueeze(2).broadcast_to([BL, J, D])
                    nc.vector.scalar_tensor_tensor(
                        vf[:], src_jd, cscale, den_in, ALU.mult, ALU.mult
                    )
                    nc.sync.dma_start(
                        out=vout_d[:], in_=vf[:].rearrange("p j d -> p (j d)")
                    )
                    break

                # v in (d, j) order bf16 for broadcast against U
                vb = small.tile([BL, D, J], BF16, tag="vb")
                den_dj = rden[:].unsqueeze(1).broadcast_to([BL, D, J])
                nc.vector.scalar_tensor_tensor(
                    vb[:], spt[:].rearrange("p (d j) -> p d j", d=D, j=J),
                    cscale, den_dj, ALU.mult, ALU.mult,
                )
                # replicate across 128 partitions via PE
                vps = vpsum.tile([128, JD], F32, tag="vps")
                nc.tensor.matmul(
                    vps[:], lhsT=bc8[:], rhs=vb[:].rearrange("p d j -> p (d j)"),
                    start=True, stop=True,
                )
                nc.vector.tensor_copy(vrep[:].rearrange("p d j -> p (d j)"), vps[:])

                # ---- sweep: agree(v_it) -> softmax -> weight -> s-matmul
                spt_i = spool.tile([BL, JD], F32, tag=f"s{it + 1}")
                for blk in range(KC // BLK):
                    eng = nc.gpsimd if blk in POOL_BLOCKS else nc.vector
                    sl = slice(blk * BLK, (blk + 1) * BLK)
                    ub = U[:, sl, :, :]
                    vin = vrep[:].unsqueeze(1).broadcast_to([128, BLK, D, J])
                    z = zpool.tile([128, BLK, D, J], BF16, tag="z")
                    eng.tensor_mul(z[:], ub, vin)
                    _stt_add(eng, z[:, :, 0:8, :], z[:, :, 0:8, :], z[:, :, 8:16, :])
                    _stt_add(eng, z[:, :, 0:4, :], z[:, :, 0:4, :], z[:, :, 4:8, :])
                    _stt_add(eng, z[:, :, 0:2, :], z[:, :, 0:2, :], z[:, :, 2:4, :])
                    if it == 0:
                        _stt_add(
                            eng, blg[:, sl, :],
                            z[:, :, 0, :], z[:, :, 1, :],
                        )
                    else:
                        t1 = zpool.tile([128, BLK, J], BF16, tag="t1")
                        _stt_add(eng, t1[:], z[:, :, 0, :], z[:, :, 1, :])
                        _stt_add(eng, blg[:, sl, :], blg[:, sl, :], t1[:])

                    # softmax over j (local per (p, kc))
                    eb = zpool.tile([128, BLK, J], BF16, tag="eb")
                    zs = zpool.tile([128, BLK], F32, tag="zs")
                    if EXP_ACCUM:
                        for q in range(BLK):
                            nc.scalar.activation(
                                eb[:, q, :], blg[:, blk * BLK + q, :], AF.Exp,
                                accum_out=zs[:, q:q + 1],
                            )
                    else:
                        nc.scalar.activation(eb[:], blg[:, sl, :], AF.Exp)
                        nc.vector.tensor_reduce(zs[:], eb[:], axis=AX.X, op=ALU.add)
                    zr = zpool.tile([128, BLK], F32, tag="zr")
                    nc.vector.reciprocal(zr[:], zs[:])
                    zin = zr[:].unsqueeze(2).broadcast_to([128, BLK, J])
                    nc.vector.tensor_mul(eb[:], eb[:], zin)

                    # weight + s-matmul
                    y = zpool.tile([128, BLK, D, J], BF16, tag="y")
                    ein = eb[:].unsqueeze(2).broadcast_to([128, BLK, D, J])
                    yeng = nc.gpsimd if blk in POOL_Y else eng
                    yeng.tensor_mul(y[:], ub, ein)
                    for q in range(BLK):
                        kc = blk * BLK + q
                        nc.tensor.matmul(
                            spt_i[:],
                            lhsT=onesbd[:],
                            rhs=y[:, q, :, :].rearrange("p d j -> p (d j)"),
                            start=(kc == 0),
                            stop=(kc == KC - 1),
                        )


_NC_CACHE = None


def _get_nc():
    global _NC_CACHE
    if _NC_CACHE is None:
        _NC_CACHE = _build_nc()
    return _NC_CACHE


def _prep_inputs(x, W):
    bf = ml_dtypes.bfloat16
    # wm[kc, (n16,i8), (d,j)] = W[j, kc*16+n16, d, i], grouped for DMA
    Wr = np.asarray(W, np.float32).reshape(J, KC, 16, D, I)
    wm = Wr.transpose(1, 2, 4, 3, 0).reshape(KC, 128, JD)
    wdt = mybir.dt.np(mybir.dt.float8e4) if WM_FP8 else bf
    wm = np.ascontiguousarray(
        wm.reshape(NG, GRP, 128, JD).transpose(0, 2, 1, 3).astype(wdt)
    )
    onesbd = np.ascontiguousarray(
        np.tile(np.eye(BL, dtype=np.float32), (16, 1)).astype(bf)
    )
    bcast8 = np.ascontiguousarray(
        np.tile(np.eye(BL, dtype=np.float32), (1, 16)).astype(bf)
    )
    # msk[(n16,i), (n16', b)] = 1 if n16' == n16
    msk = np.zeros((16, I, 16, BL), np.float32)
    idx = np.arange(16)
    msk[idx, :, idx, :] = 1.0
    msk = np.ascontiguousarray(msk.reshape(128, 128).astype(bf))
    in_maps = []
    xr = np.asarray(x, np.float32).reshape(NCORES, BL, KC, 16, I)
    for c in range(NCORES):
        # xs[(n16,i8), kc, b] = x[c, b, kc, n16, i]
        xsc = np.ascontiguousarray(
            xr[c].transpose(2, 3, 1, 0)            # [n16, i, kc, b]
            .reshape(128, KC, BL)
            .reshape(128, NG, GRP, BL)
            .astype(bf)
        )
        m = {"xs": xsc, "wm": wm, "onesbd": onesbd, "bcast8": bcast8}
        if ONCHIP_XBD:
            m["msk"] = msk
        else:
            xbd = np.zeros((KC, 16, I, 16, BL), np.float32)
            xbd[:, idx, :, idx, :] = xr[c].transpose(2, 1, 3, 0)
            xbd = xbd.reshape(KC, 128, 128)
            m["xbd"] = np.ascontiguousarray(
                xbd.reshape(NG, GRP, 128, 128).transpose(0, 2, 1, 3).astype(bf)
            )
        in_maps.append(m)
    return in_maps


def kernel(x, W):
    nc = _get_nc()
    in_maps = _prep_inputs(x, W)
    res = run_bass_kernel_spmd(nc, in_maps, list(range(NCORES)))
    outs = [res.results[c]["vout"].reshape(BL, J, D) for c in range(NCORES)]
    return np.concatenate(outs, axis=0).astype(np.float32)


# revision 7
# speedup vs baseline: 1.1196x; 1.1196x over previous
"""CapsuleLayer dynamic-routing kernel v2 for Trainium2 (8 NeuronCores).

Data-parallel over batch B (64 -> 8 per core), zero communication.

Layout: U[p=(n16,b8), kc, (d,j)] bf16 in SBUF; psum cols are (d,j) so
drains are contiguous. s0 (uniform-c weighted sum) accumulates on the PE
during phase 1 via a dense-x stationary. Routing sweeps are block-
pipelined: agree (TT mul + d-tree adds) -> softmax (per-kc exp with
fused Z-accumulation on Act, recip, norm) -> weight-mul -> s-matmul on
PE. All tensor-tensor work stays on DVE (real GpSimd is far slower than
the cost model claims); Act handles drains, exp, square/ln/exp squash
from one pinned activation table.
"""

import numpy as np
import ml_dtypes

from concourse import bass
import concourse.mybir as mybir
import concourse.bacc as bacc
import concourse.tile as tile
from concourse.bass_utils import run_bass_kernel_spmd

BF16 = mybir.dt.bfloat16
F32 = mybir.dt.float32
AF = mybir.ActivationFunctionType
ALU = mybir.AluOpType
AX = mybir.AxisListType

B, N, I, J, D = 64, 2048, 8, 32, 16
NCORES = 8
BL = B // NCORES          # 8 local batches
KC = N // 16              # 128 contraction chunks of 16 n's
JD = J * D                # 512
GRP = 4                   # kc's per DMA batch
NG = KC // GRP            # 32
BLK = 8                   # kc's per routing block
NBLK = KC // BLK          # 16
ZBUFS = 2
EPS = 1e-7


POOL_BLOCKS = ()
DRAIN_PATTERN = "vsvs"
ONCHIP_XBD = False
WM_FP8 = False
EXP_ACCUM = True
POOL_Y = ()
DIAG_Z = False


def _build_nc(reps=1):
    nc = bacc.Bacc("TRN2", target_bir_lowering=False)
    if ONCHIP_XBD:
        xbd_d = None
        msk_d = nc.declare_dram_parameter("msk", [128, 128], BF16, False)
    else:
        xbd_d = nc.declare_dram_parameter("xbd", [NG, 128, GRP, 128], BF16, False)
        msk_d = None
    xs_d = nc.declare_dram_parameter("xs", [128, NG, GRP, BL], BF16, False)
    wm_d = nc.declare_dram_parameter(
        "wm", [NG, 128, GRP, JD], mybir.dt.float8e4 if WM_FP8 else BF16, False
    )
    ones_d = nc.declare_dram_parameter("onesbd", [128, BL], BF16, False)
    bc8_d = nc.declare_dram_parameter("bcast8", [BL, 128], BF16, False)
    vout_d = nc.declare_dram_parameter("vout", [BL, JD], F32, True)

    for _ in range(reps):
        _emit_body(nc, xbd_d, msk_d, xs_d, wm_d, ones_d, bc8_d, vout_d)
    nc.compile()
    return nc


def _stt_add(eng, out, a, b):
    eng.tensor_add(out, a, b)


def _emit_body(nc, xbd_d, msk_d, xs_d, wm_d, ones_d, bc8_d, vout_d):
    with tile.TileContext(nc) as tc:
        with (
            tc.tile_pool(name="big", bufs=1) as big,
            tc.tile_pool(name="wpool", bufs=3) as wpool,
            tc.tile_pool(name="xpool", bufs=3) as xpool,
            tc.tile_pool(name="ppool", bufs=4, space="PSUM") as ppool,
            tc.tile_pool(name="spool", bufs=1, space="PSUM") as spool,
            tc.tile_pool(name="vpool", bufs=1, space="PSUM") as vpsum,
            tc.tile_pool(name="zpool", bufs=ZBUFS) as zpool,
            tc.tile_pool(name="small", bufs=1) as small,
        ):
            # persistent SBUF
            U = big.tile([128, KC, D, J], BF16, tag="U")
            blg = big.tile([128, KC, J], BF16, tag="blg")
            onesbd = small.tile([128, BL], BF16, tag="ones")
            xs = small.tile([128, NG, GRP, BL], BF16, tag="xs")
            bc8 = small.tile([BL, 128], BF16, tag="bc8")
            vrep = small.tile([128, D, J], BF16, tag="vrep")
            epst = small.tile([128, 1], F32, tag="epst")
            nc.vector.memset(epst[:], EPS)
            # pin the act table to natural_log_exp_and_others (ln+exp+square
            # +copy) so no reloads happen mid-kernel
            actwarm = small.tile([128, 1], F32, tag="actwarm")
            nc.scalar.activation(actwarm[:], epst[:], AF.Ln)
            if ONCHIP_XBD:
                msk = small.tile([128, 128], BF16, tag="msk")
                nc.sync.dma_start(out=msk[:], in_=msk_d[:])
            nc.sync.dma_start(out=onesbd[:], in_=ones_d[:])
            nc.sync.dma_start(out=bc8[:], in_=bc8_d[:])
            nc.sync.dma_start(out=xs[:], in_=xs_d[:])

            # ---- Phase A: u_hat + s0 accumulation ----
            spt0 = spool.tile([BL, JD], F32, tag="s0")
            drain_eng = [{"v": nc.vector, "s": nc.scalar}[c] for c in DRAIN_PATTERN]
            for g in range(NG):
                wt = wpool.tile(
                    [128, GRP, JD], mybir.dt.float8e4 if WM_FP8 else BF16, tag="w"
                )
                xt = xpool.tile([128, GRP, 128], BF16, tag="x")
                nc.sync.dma_start(out=wt[:], in_=wm_d[g])
                if ONCHIP_XBD:
                    xsin = (
                        xs[:, g, :, :]
                        .unsqueeze(2)
                        .broadcast_to([128, GRP, 16, BL])
                    )
                    mskin = (
                        msk[:]
                        .rearrange("p (n b) -> p n b", n=16, b=BL)
                        .unsqueeze(1)
                        .broadcast_to([128, GRP, 16, BL])
                    )
                    xtv = xt[:].rearrange("p q (n b) -> p q n b", n=16, b=BL)
                    nc.vector.tensor_mul(xtv, xsin, mskin)
                else:
                    nc.sync.dma_start(out=xt[:], in_=xbd_d[g])
                for q in range(GRP):
                    kc = g * GRP + q
                    pt = ppool.tile([128, JD], F32, tag="p1")
                    nc.tensor.matmul(
                        pt[:], lhsT=xt[:, q, :], rhs=wt[:, q, :],
                        start=True, stop=True,
                    )
                    nc.tensor.matmul(
                        spt0[:], lhsT=xs[:, g, q, :], rhs=wt[:, q, :],
                        start=(kc == 0), stop=(kc == KC - 1),
                    )
                    eng = drain_eng[kc % 4]
                    dst = U[:, kc, :, :].rearrange("p d j -> p (d j)")
                    if eng is nc.scalar:
                        eng.copy(dst, pt[:])
                    else:
                        eng.tensor_copy(dst, pt[:])

            # ---- routing ----
            for it in range(3):
                # squash s -> v; v in psum row-broadcast to vrep
                spt = spt0 if it == 0 else spt_i
                cscale = (1.0 / J) if it == 0 else 1.0
                # sq[b, j, d] = (s * cscale)^2 straight from PSUM on Act
                sq = small.tile([BL, J, D], F32, tag="sq")
                src_jd = spt[:].rearrange("p (d j) -> p j d", d=D, j=J)
                nc.scalar.activation(sq[:], src_jd, AF.Square, scale=cscale)
                ssq = small.tile([BL, J], F32, tag="ssq")
                nc.vector.tensor_reduce(ssq[:], sq[:], axis=AX.X, op=ALU.add)
                # rden = (ssq + eps)^-0.5 via ln/exp (same act table as Exp)
                lnq = small.tile([BL, J], F32, tag="lnq")
                nc.scalar.activation(lnq[:], ssq[:], AF.Ln, bias=epst[0:BL, :])
                rden = small.tile([BL, J], F32, tag="rden")
                nc.scalar.activation(rden[:], lnq[:], AF.Exp, scale=-0.5)

                if it == 2:
                    # final output v[b, j, d] fp32 = s * cscale * rden
                    vf = small.tile([BL, J, D], F32, tag="vf")
                    den_in = rden[:].unsqueeze(2).broadcast_to([BL, J, D])
                    nc.vector.scalar_tensor_tensor(
                        vf[:], src_jd, cscale, den_in, ALU.mult, ALU.mult
                    )
                    nc.sync.dma_start(
                        out=vout_d[:], in_=vf[:].rearrange("p j d -> p (j d)")
                    )
                    break

                # v in (d, j) order bf16 for broadcast against U
                vb = small.tile([BL, D, J], BF16, tag="vb")
                den_dj = rden[:].unsqueeze(1).broadcast_to([BL, D, J])
                nc.vector.scalar_tensor_tensor(
                    vb[:], spt[:].rearrange("p (d j) -> p d j", d=D, j=J),
                    cscale, den_dj, ALU.mult, ALU.mult,
                )
                # replicate across 128 partitions via PE
                vps = vpsum.tile([128, JD], F32, tag="vps")
                nc.tensor.matmul(
                    vps[:], lhsT=bc8[:], rhs=vb[:].rearrange("p d j -> p (d j)"),
                    start=True, stop=True,
                )
                nc.vector.tensor_copy(vrep[:].rearrange("p d j -> p (d j)"), vps[:])

                # ---- sweep: agree(v_it) -> softmax -> weight -> s-matmul
                spt_i = spool.tile([BL, JD], F32, tag=f"s{it + 1}")
                for blk in range(KC // BLK):
                    eng = nc.gpsimd if blk in POOL_BLOCKS else nc.vector
                    sl = slice(blk * BLK, (blk + 1) * BLK)
                    ub = U[:, sl, :, :]
                    vin = vrep[:].unsqueeze(1).broadcast_to([128, BLK, D, J])
                    z = zpool.tile([128, BLK, D, J], BF16, tag="z")
                    eng.tensor_mul(z[:], ub, vin)
                    _stt_add(eng, z[:, :, 0:8, :], z[:, :, 0:8, :], z[:, :, 8:16, :])
                    _stt_add(eng, z[:, :, 0:4, :], z[:, :, 0:4, :], z[:, :, 4:8, :])
                    _stt_add(eng, z[:, :, 0:2, :], z[:, :, 0:2, :], z[:, :, 2:4, :])
                    if it == 0:
                        _stt_add(
                            eng, blg[:, sl, :],
                            z[:, :, 0, :], z[:, :, 1, :],
                        )
                    else:
                        t1 = zpool.tile([128, BLK, J], BF16, tag="t1")
                        _stt_add(eng, t1[:], z[:, :, 0, :], z[:, :, 1, :])
                        _stt_add(eng, blg[:, sl, :], blg[:, sl, :], t1[:])

                    # softmax over j (local per (p, kc))
                    eb = zpool.tile([128, BLK, J], BF16, tag="eb")
                    zs = zpool.tile([128, BLK], F32, tag="zs")
                    if EXP_ACCUM:
                        for q in range(BLK):
                            nc.scalar.activation(
                                eb[:, q, :], blg[:, blk * BLK + q, :], AF.Exp,
                                accum_out=zs[:, q:q + 1],
                            )
                    else:
                        nc.scalar.activation(eb[:], blg[:, sl, :], AF.Exp)
                        nc.vector.tensor_reduce(zs[:], eb[:], axis=AX.X, op=ALU.add)
                    zr = zpool.tile([128, BLK], F32, tag="zr")
                    nc.vector.reciprocal(zr[:], zs[:])
                    if DIAG_Z:
                        # fold 1/Z into the s-matmul stationary: per-kc
                        # diagonal delta_{b,b'} * zr[p, kc]
                        zd = zpool.tile([128, BLK, BL], BF16, tag="zd")
                        zin = zr[:].unsqueeze(2).broadcast_to([128, BLK, BL])
                        obin = onesbd[:].unsqueeze(1).broadcast_to([128, BLK, BL])
                        nc.vector.tensor_mul(zd[:], obin, zin)
                    else:
                        zin = zr[:].unsqueeze(2).broadcast_to([128, BLK, J])
                        nc.vector.tensor_mul(eb[:], eb[:], zin)

                    # weight + s-matmul
                    y = zpool.tile([128, BLK, D, J], BF16, tag="y")
                    ein = eb[:].unsqueeze(2).broadcast_to([128, BLK, D, J])
                    yeng = nc.gpsimd if blk in POOL_Y else eng
                    yeng.tensor_mul(y[:], ub, ein)
                    for q in range(BLK):
                        kc = blk * BLK + q
                        nc.tensor.matmul(
                            spt_i[:],
                            lhsT=zd[:, q, :] if DIAG_Z else onesbd[:],
                            rhs=y[:, q, :, :].rearrange("p d j -> p (d j)"),
                            start=(kc == 0),
                            stop=(kc == KC - 1),
                        )


_NC_CACHE = None


def _get_nc():
    global _NC_CACHE
    if _NC_CACHE is None:
        _NC_CACHE = _build_nc()
    return _NC_CACHE


def _prep_inputs(x, W):
    bf = ml_dtypes.bfloat16
    # wm[kc, (n16,i8), (d,j)] = W[j, kc*16+n16, d, i], grouped for DMA
    Wr = np.asarray(W, np.float32).reshape(J, KC, 16, D, I)
    wm = Wr.transpose(1, 2, 4, 3, 0).reshape(KC, 128, JD)
    wdt = mybir.dt.np(mybir.dt.float8e4) if WM_FP8 else bf
    wm = np.ascontiguousarray(
        wm.reshape(NG, GRP, 128, JD).transpose(0, 2, 1, 3).astype(wdt)
    )
    onesbd = np.ascontiguousarray(
        np.tile(np.eye(BL, dtype=np.float32), (16, 1)).astype(bf)
    )
    bcast8 = np.ascontiguousarray(
        np.tile(np.eye(BL, dtype=np.float32), (1, 16)).astype(bf)
    )
    # msk[(n16,i), (n16', b)] = 1 if n16' == n16
    msk = np.zeros((16, I, 16, BL), np.float32)
    idx = np.arange(16)
    msk[idx, :, idx, :] = 1.0
    msk = np.ascontiguousarray(msk.reshape(128, 128).astype(bf))
    in_maps = []
    xr = np.asarray(x, np.float32).reshape(NCORES, BL, KC, 16, I)
    for c in range(NCORES):
        # xs[(n16,i8), kc, b] = x[c, b, kc, n16, i]
        xsc = np.ascontiguousarray(
            xr[c].transpose(2, 3, 1, 0)            # [n16, i, kc, b]
            .reshape(128, KC, BL)
            .reshape(128, NG, GRP, BL)
            .astype(bf)
        )
        m = {"xs": xsc, "wm": wm, "onesbd": onesbd, "bcast8": bcast8}
        if ONCHIP_XBD:
            m["msk"] = msk
        else:
            xbd = np.zeros((KC, 16, I, 16, BL), np.float32)
            xbd[:, idx, :, idx, :] = xr[c].transpose(2, 1, 3, 0)
            xbd = xbd.reshape(KC, 128, 128)
            m["xbd"] = np.ascontiguousarray(
                xbd.reshape(NG, GRP, 128, 128).transpose(0, 2, 1, 3).astype(bf)
            )
        in_maps.append(m)
    return in_maps


def kernel(x, W):
    nc = _get_nc()
    in_maps = _prep_inputs(x, W)
    res = run_bass_kernel_spmd(nc, in_maps, list(range(NCORES)))
    outs = [res.results[c]["vout"].reshape(BL, J, D) for c in range(NCORES)]
    return np.concatenate(outs, axis=0).astype(np.float32)


# revision 8
# speedup vs baseline: 1.1715x; 1.0464x over previous
"""CapsuleLayer dynamic-routing kernel v2 for Trainium2 (8 NeuronCores).

Data-parallel over batch B (64 -> 8 per core), zero communication.

Layout: U[p=(n16,b8), kc, (d,j)] bf16 in SBUF; psum cols are (d,j) so
drains are contiguous. s0 (uniform-c weighted sum) accumulates on the PE
during phase 1 via a dense-x stationary. Routing sweeps are block-
pipelined: agree (TT mul + d-tree adds) -> softmax (per-kc exp with
fused Z-accumulation on Act, recip, norm) -> weight-mul -> s-matmul on
PE. All tensor-tensor work stays on DVE (real GpSimd is far slower than
the cost model claims); Act handles drains, exp, square/ln/exp squash
from one pinned activation table.
"""

import numpy as np
import ml_dtypes

from concourse import bass
import concourse.mybir as mybir
import concourse.bacc as bacc
import concourse.tile as tile
from concourse.bass_utils import run_bass_kernel_spmd

BF16 = mybir.dt.bfloat16
F32 = mybir.dt.float32
AF = mybir.ActivationFunctionType
ALU = mybir.AluOpType
AX = mybir.AxisListType

B, N, I, J, D = 64, 2048, 8, 32, 16
NCORES = 8
BL = B // NCORES          # 8 local batches
KC = N // 16              # 128 contraction chunks of 16 n's
JD = J * D                # 512
GRP = 4                   # kc's per DMA batch
NG = KC // GRP            # 32
BLK = 8                   # kc's per routing block
NBLK = KC // BLK          # 16
ZBUFS = 2
EPS = 1e-7


POOL_BLOCKS = ()
DRAIN_PATTERN = "vsvs"
ONCHIP_XBD = False
WM_FP8 = False
EXP_ACCUM = True
POOL_Y = ()
DIAG_Z = False
S0_ONES = False


def _build_nc(reps=1):
    nc = bacc.Bacc("TRN2", target_bir_lowering=False)
    if ONCHIP_XBD:
        xbd_d = None
        msk_d = nc.declare_dram_parameter("msk", [128, 128], BF16, False)
    else:
        xbd_d = nc.declare_dram_parameter("xbd", [NG, 128, GRP, 128], BF16, False)
        msk_d = None
    xs_d = nc.declare_dram_parameter("xs", [128, NG, GRP, BL], BF16, False)
    wm_d = nc.declare_dram_parameter(
        "wm", [NG, 128, GRP, JD], mybir.dt.float8e4 if WM_FP8 else BF16, False
    )
    ones_d = nc.declare_dram_parameter("onesbd", [128, BL], BF16, False)
    bc8_d = nc.declare_dram_parameter("bcast8", [BL, 128], BF16, False)
    vout_d = nc.declare_dram_parameter("vout", [BL, JD], F32, True)

    for _ in range(reps):
        _emit_body(nc, xbd_d, msk_d, xs_d, wm_d, ones_d, bc8_d, vout_d)
    nc.compile()
    return nc


def _stt_add(eng, out, a, b):
    eng.tensor_add(out, a, b)


def _emit_body(nc, xbd_d, msk_d, xs_d, wm_d, ones_d, bc8_d, vout_d):
    with tile.TileContext(nc) as tc:
        with (
            tc.tile_pool(name="big", bufs=1) as big,
            tc.tile_pool(name="wpool", bufs=3) as wpool,
            tc.tile_pool(name="xpool", bufs=3) as xpool,
            tc.tile_pool(name="ppool", bufs=4, space="PSUM") as ppool,
            tc.tile_pool(name="spool", bufs=1, space="PSUM") as spool,
            tc.tile_pool(name="vpool", bufs=1, space="PSUM") as vpsum,
            tc.tile_pool(name="zpool", bufs=ZBUFS) as zpool,
            tc.tile_pool(name="small", bufs=1) as small,
        ):
            # persistent SBUF
            U = big.tile([128, KC, D, J], BF16, tag="U")
            blg = big.tile([128, KC, J], BF16, tag="blg")
            onesbd = small.tile([128, BL], BF16, tag="ones")
            xs = small.tile([128, NG, GRP, BL], BF16, tag="xs")
            bc8 = small.tile([BL, 128], BF16, tag="bc8")
            vrep = small.tile([128, D, J], BF16, tag="vrep")
            epst = small.tile([128, 1], F32, tag="epst")
            nc.vector.memset(epst[:], EPS)
            # pin the act table to natural_log_exp_and_others (ln+exp+square
            # +copy) so no reloads happen mid-kernel
            actwarm = small.tile([128, 1], F32, tag="actwarm")
            nc.scalar.activation(actwarm[:], epst[:], AF.Ln)
            if ONCHIP_XBD:
                msk = small.tile([128, 128], BF16, tag="msk")
                nc.sync.dma_start(out=msk[:], in_=msk_d[:])
            nc.sync.dma_start(out=onesbd[:], in_=ones_d[:])
            nc.sync.dma_start(out=bc8[:], in_=bc8_d[:])
            nc.sync.dma_start(out=xs[:], in_=xs_d[:])

            # ---- Phase A: u_hat + s0 accumulation ----
            spt0 = spool.tile([BL, JD], F32, tag="s0")
            drain_eng = [{"v": nc.vector, "s": nc.scalar}[c] for c in DRAIN_PATTERN]
            for g in range(NG):
                wt = wpool.tile(
                    [128, GRP, JD], mybir.dt.float8e4 if WM_FP8 else BF16, tag="w"
                )
                xt = xpool.tile([128, GRP, 128], BF16, tag="x")
                nc.sync.dma_start(out=wt[:], in_=wm_d[g])
                if ONCHIP_XBD:
                    xsin = (
                        xs[:, g, :, :]
                        .unsqueeze(2)
                        .broadcast_to([128, GRP, 16, BL])
                    )
                    mskin = (
                        msk[:]
                        .rearrange("p (n b) -> p n b", n=16, b=BL)
                        .unsqueeze(1)
                        .broadcast_to([128, GRP, 16, BL])
                    )
                    xtv = xt[:].rearrange("p q (n b) -> p q n b", n=16, b=BL)
                    nc.vector.tensor_mul(xtv, xsin, mskin)
                else:
                    nc.sync.dma_start(out=xt[:], in_=xbd_d[g])
                for q in range(GRP):
                    kc = g * GRP + q
                    pt = ppool.tile([128, JD], F32, tag="p1")
                    nc.tensor.matmul(
                        pt[:], lhsT=xt[:, q, :], rhs=wt[:, q, :],
                        start=True, stop=True,
                    )
                    if not S0_ONES:
                        nc.tensor.matmul(
                            spt0[:], lhsT=xs[:, g, q, :], rhs=wt[:, q, :],
                            start=(kc == 0), stop=(kc == KC - 1),
                        )
                    eng = drain_eng[kc % 4]
                    dst = U[:, kc, :, :].rearrange("p d j -> p (d j)")
                    if eng is nc.scalar:
                        eng.copy(dst, pt[:])
                    else:
                        eng.tensor_copy(dst, pt[:])
                if S0_ONES:
                    # s0 += sum_n u_hat via shared-stationary ones chain,
                    # batched after the group's drains (one Ldweights source)
                    for q in range(GRP):
                        kc = g * GRP + q
                        nc.tensor.matmul(
                            spt0[:],
                            lhsT=onesbd[:],
                            rhs=U[:, kc, :, :].rearrange("p d j -> p (d j)"),
                            start=(kc == 0), stop=(kc == KC - 1),
                        )

            # ---- routing ----
            for it in range(3):
                # squash s -> v; v in psum row-broadcast to vrep
                spt = spt0 if it == 0 else spt_i
                cscale = (1.0 / J) if it == 0 else 1.0
                # sq[b, j, d] = (s * cscale)^2 straight from PSUM on Act
                sq = small.tile([BL, J, D], F32, tag="sq")
                src_jd = spt[:].rearrange("p (d j) -> p j d", d=D, j=J)
                nc.scalar.activation(sq[:], src_jd, AF.Square, scale=cscale)
                ssq = small.tile([BL, J], F32, tag="ssq")
                nc.vector.tensor_reduce(ssq[:], sq[:], axis=AX.X, op=ALU.add)
                # rden = (ssq + eps)^-0.5 via ln/exp (same act table as Exp)
                lnq = small.tile([BL, J], F32, tag="lnq")
                nc.scalar.activation(lnq[:], ssq[:], AF.Ln, bias=epst[0:BL, :])
                rden = small.tile([BL, J], F32, tag="rden")
                nc.scalar.activation(rden[:], lnq[:], AF.Exp, scale=-0.5)

                if it == 2:
                    # final output v[b, j, d] fp32 = s * cscale * rden
                    vf = small.tile([BL, J, D], F32, tag="vf")
                    den_in = rden[:].unsqueeze(2).broadcast_to([BL, J, D])
                    nc.vector.scalar_tensor_tensor(
                        vf[:], src_jd, cscale, den_in, ALU.mult, ALU.mult
                    )
                    nc.sync.dma_start(
                        out=vout_d[:], in_=vf[:].rearrange("p j d -> p (j d)")
                    )
                    break

                # v in (d, j) order bf16 for broadcast against U
                vb = small.tile([BL, D, J], BF16, tag="vb")
                den_dj = rden[:].unsqueeze(1).broadcast_to([BL, D, J])
                nc.vector.scalar_tensor_tensor(
                    vb[:], spt[:].rearrange("p (d j) -> p d j", d=D, j=J),
                    cscale, den_dj, ALU.mult, ALU.mult,
                )
                # replicate across 128 partitions via PE
                vps = vpsum.tile([128, JD], F32, tag="vps")
                nc.tensor.matmul(
                    vps[:], lhsT=bc8[:], rhs=vb[:].rearrange("p d j -> p (d j)"),
                    start=True, stop=True,
                )
                nc.vector.tensor_copy(vrep[:].rearrange("p d j -> p (d j)"), vps[:])

                # ---- sweep: agree(v_it) -> softmax -> weight -> s-matmul
                spt_i = spool.tile([BL, JD], F32, tag=f"s{it + 1}")
                for blk in range(KC // BLK):
                    eng = nc.gpsimd if blk in POOL_BLOCKS else nc.vector
                    sl = slice(blk * BLK, (blk + 1) * BLK)
                    ub = U[:, sl, :, :]
                    vin = vrep[:].unsqueeze(1).broadcast_to([128, BLK, D, J])
                    z = zpool.tile([128, BLK, D, J], BF16, tag="z")
                    eng.tensor_mul(z[:], ub, vin)
                    _stt_add(eng, z[:, :, 0:8, :], z[:, :, 0:8, :], z[:, :, 8:16, :])
                    _stt_add(eng, z[:, :, 0:4, :], z[:, :, 0:4, :], z[:, :, 4:8, :])
                    _stt_add(eng, z[:, :, 0:2, :], z[:, :, 0:2, :], z[:, :, 2:4, :])
                    if it == 0:
                        _stt_add(
                            eng, blg[:, sl, :],
                            z[:, :, 0, :], z[:, :, 1, :],
                        )
                    else:
                        t1 = zpool.tile([128, BLK, J], BF16, tag="t1")
                        _stt_add(eng, t1[:], z[:, :, 0, :], z[:, :, 1, :])
                        _stt_add(eng, blg[:, sl, :], blg[:, sl, :], t1[:])

                    # softmax over j (local per (p, kc))
                    eb = zpool.tile([128, BLK, J], BF16, tag="eb")
                    zs = zpool.tile([128, BLK], F32, tag="zs")
                    if EXP_ACCUM:
                        for q in range(BLK):
                            nc.scalar.activation(
                                eb[:, q, :], blg[:, blk * BLK + q, :], AF.Exp,
                                accum_out=zs[:, q:q + 1],
                            )
                    else:
                        nc.scalar.activation(eb[:], blg[:, sl, :], AF.Exp)
                        nc.vector.tensor_reduce(zs[:], eb[:], axis=AX.X, op=ALU.add)
                    zr = zpool.tile([128, BLK], F32, tag="zr")
                    nc.vector.reciprocal(zr[:], zs[:])
                    if DIAG_Z:
                        # fold 1/Z into the s-matmul stationary: per-kc
                        # diagonal delta_{b,b'} * zr[p, kc]
                        zd = zpool.tile([128, BLK, BL], BF16, tag="zd")
                        zin = zr[:].unsqueeze(2).broadcast_to([128, BLK, BL])
                        obin = onesbd[:].unsqueeze(1).broadcast_to([128, BLK, BL])
                        nc.vector.tensor_mul(zd[:], obin, zin)
                    else:
                        zin = zr[:].unsqueeze(2).broadcast_to([128, BLK, J])
                        nc.vector.tensor_mul(eb[:], eb[:], zin)

                    # weight + s-matmul
                    y = zpool.tile([128, BLK, D, J], BF16, tag="y")
                    ein = eb[:].unsqueeze(2).broadcast_to([128, BLK, D, J])
                    yeng = nc.gpsimd if blk in POOL_Y else eng
                    yeng.tensor_mul(y[:], ub, ein)
                    for q in range(BLK):
                        kc = blk * BLK + q
                        nc.tensor.matmul(
                            spt_i[:],
                            lhsT=zd[:, q, :] if DIAG_Z else onesbd[:],
                            rhs=y[:, q, :, :].rearrange("p d j -> p (d j)"),
                            start=(kc == 0),
                            stop=(kc == KC - 1),
                        )


_NC_CACHE = None


def _get_nc():
    global _NC_CACHE
    if _NC_CACHE is None:
        _NC_CACHE = _build_nc()
    return _NC_CACHE


def _prep_inputs(x, W):
    bf = ml_dtypes.bfloat16
    # wm[kc, (n16,i8), (d,j)] = W[j, kc*16+n16, d, i], grouped for DMA
    Wr = np.asarray(W, np.float32).reshape(J, KC, 16, D, I)
    wm = Wr.transpose(1, 2, 4, 3, 0).reshape(KC, 128, JD)
    wdt = mybir.dt.np(mybir.dt.float8e4) if WM_FP8 else bf
    wm = np.ascontiguousarray(
        wm.reshape(NG, GRP, 128, JD).transpose(0, 2, 1, 3).astype(wdt)
    )
    onesbd = np.ascontiguousarray(
        np.tile(np.eye(BL, dtype=np.float32), (16, 1)).astype(bf)
    )
    bcast8 = np.ascontiguousarray(
        np.tile(np.eye(BL, dtype=np.float32), (1, 16)).astype(bf)
    )
    # msk[(n16,i), (n16', b)] = 1 if n16' == n16
    msk = np.zeros((16, I, 16, BL), np.float32)
    idx = np.arange(16)
    msk[idx, :, idx, :] = 1.0
    msk = np.ascontiguousarray(msk.reshape(128, 128).astype(bf))
    in_maps = []
    xr = np.asarray(x, np.float32).reshape(NCORES, BL, KC, 16, I)
    for c in range(NCORES):
        # xs[(n16,i8), kc, b] = x[c, b, kc, n16, i]
        xsc = np.ascontiguousarray(
            xr[c].transpose(2, 3, 1, 0)            # [n16, i, kc, b]
            .reshape(128, KC, BL)
            .reshape(128, NG, GRP, BL)
            .astype(bf)
        )
        m = {"xs": xsc, "wm": wm, "onesbd": onesbd, "bcast8": bcast8}
        if ONCHIP_XBD:
            m["msk"] = msk
        else:
            xbd = np.zeros((KC, 16, I, 16, BL), np.float32)
            xbd[:, idx, :, idx, :] = xr[c].transpose(2, 1, 3, 0)
            xbd = xbd.reshape(KC, 128, 128)
            m["xbd"] = np.ascontiguousarray(
                xbd.reshape(NG, GRP, 128, 128).transpose(0, 2, 1, 3).astype(bf)
            )
        in_maps.append(m)
    return in_maps


def kernel(x, W):
    nc = _get_nc()
    in_maps = _prep_inputs(x, W)
    res = run_bass_kernel_spmd(nc, in_maps, list(range(NCORES)))
    outs = [res.results[c]["vout"].reshape(BL, J, D) for c in range(NCORES)]
    return np.concatenate(outs, axis=0).astype(np.float32)
